# revision 12
# baseline (speedup 1.0000x reference)
"""CrossAttentionBlock3D on 8 TRN2 NeuronCores — sequence-parallel Bass kernel.

Sharding: the 32768 spatial tokens are split 8x4096 across cores. GroupNorm
statistics are the only cross-core dependency (one 64-float AllReduce).
Everything else (LN, K/V projections over the tiny context, Q/attention/proj
for the local tokens) is computed locally; context-side work is replicated.

Compute dtype: bf16 matmuls with f32 PSUM accumulation (validated end-to-end
rel err ~3e-3 vs the f32 reference).
"""
import sys

sys.path.insert(0, "/opt/trn_rl_repo")

import numpy as np
import ml_dtypes

from concourse import bass, bacc, tile, mybir, masks
from concourse.bass_utils import run_bass_kernel_spmd

F32 = mybir.dt.float32
BF16 = mybir.dt.bfloat16
BF = ml_dtypes.bfloat16

B, C, D, H, W = 2, 512, 32, 32, 32
S = D * H * W              # 32768
L, CTX = 256, 768
NH, HD, G = 8, 64, 8
EPS = 1e-5
NCORES = 8
CT, OT = 4, 4              # channel tiles (C = 4*128)
KCT = 6                    # ctx channel tiles (CTX = 6*128)
LT = 2                     # L = 2*128


def build(nc, s_loc, st_size):
    """Emit the per-core Tile program. s_loc: local tokens; st_size: S-tile."""
    nst = s_loc // st_size
    n_gn = (C // G) * S    # global elems per (b, g) group

    x_in = nc.dram_tensor("x", [B, CT, 128, s_loc], BF16, kind="ExternalInput")
    ctx_in = nc.dram_tensor("ctx", [B, LT, 128, CTX], F32, kind="ExternalInput")
    qw_in = nc.dram_tensor("qwT", [CT, 128, C], BF16, kind="ExternalInput")
    kw_in = nc.dram_tensor("kwT", [KCT, 128, C], BF16, kind="ExternalInput")
    vw_in = nc.dram_tensor("vwT", [KCT, 128, C], BF16, kind="ExternalInput")
    pw_in = nc.dram_tensor("pwT", [CT, 128, C], BF16, kind="ExternalInput")
    qb_in = nc.dram_tensor("qb", [CT, 128, 1], F32, kind="ExternalInput")
    kb_in = nc.dram_tensor("kb", [CT, 128, 1], F32, kind="ExternalInput")
    pb_in = nc.dram_tensor("pb", [CT, 128, 1], F32, kind="ExternalInput")
    gnw_in = nc.dram_tensor("gnw8", [128, B * CT], F32, kind="ExternalInput")
    gnb_in = nc.dram_tensor("gnb8", [128, B * CT], F32, kind="ExternalInput")
    out_ext = nc.dram_tensor("out", [B, CT, 128, s_loc], F32, kind="ExternalOutput")

    from contextlib import ExitStack
    with tile.TileContext(nc) as tc, ExitStack() as es:
        wp = es.enter_context(tc.tile_pool(name="wp", bufs=1))
        dram = es.enter_context(tc.tile_pool(name="dram", bufs=1, space="DRAM"))

        # ---- persistent SBUF tensors ----
        x_all = wp.tile([128, B * CT * s_loc], BF16, tag="x_all")
        qw_t = wp.tile([128, CT * C], BF16, tag="qw")
        kw_t = wp.tile([128, KCT * C], BF16, tag="kw")
        vw_t = wp.tile([128, KCT * C], BF16, tag="vw")
        pw_t = wp.tile([128, CT * C], BF16, tag="pw")
        qb_t = wp.tile([128, CT], F32, tag="qb")
        kb_t = wp.tile([128, CT], F32, tag="kb")
        pb_t = wp.tile([128, CT], F32, tag="pb")
        gnw_t = wp.tile([128, B * CT], F32, tag="gnw")
        gnb_t = wp.tile([128, B * CT], F32, tag="gnb")
        ctxT_all = wp.tile([128, B * KCT * L], BF16, tag="ctxT")
        kT_all = wp.tile([128, B * CT * L], BF16, tag="kT")
        v_all = wp.tile([128, B * LT * (NH * (HD + 1))], BF16, tag="v")
        ones_t = wp.tile([1, 64], F32, tag="ones")
        ident = wp.tile([128, 128], BF16, tag="ident")
        stats_s = wp.tile([128, 16], F32, tag="stats")
        a_pc = wp.tile([128, B * CT], F32, tag="a_pc")
        bias_pc = wp.tile([128, B * CT], F32, tag="bias_pc")

        for b in range(B):
            for t in range(CT):
                nc.sync.dma_start(
                    x_all[:, (b * CT + t) * s_loc:(b * CT + t + 1) * s_loc],
                    x_in[b, t],
                )
        for t in range(CT):
            nc.sync.dma_start(qw_t[:, t * C:(t + 1) * C], qw_in[t])
            nc.sync.dma_start(pw_t[:, t * C:(t + 1) * C], pw_in[t])
            nc.sync.dma_start(qb_t[:, t:t + 1], qb_in[t])
            nc.sync.dma_start(kb_t[:, t:t + 1], kb_in[t])
            nc.sync.dma_start(pb_t[:, t:t + 1], pb_in[t])
        for t in range(KCT):
            nc.sync.dma_start(kw_t[:, t * C:(t + 1) * C], kw_in[t])
            nc.sync.dma_start(vw_t[:, t * C:(t + 1) * C], vw_in[t])
        nc.sync.dma_start(gnw_t[:], gnw_in[:])
        nc.sync.dma_start(gnb_t[:], gnb_in[:])
        nc.vector.memset(ones_t[:], 1.0)
        masks.make_identity(nc, ident[:])

        # ---- GroupNorm partial stats (local) ----
        with tc.tile_pool(name="setup", bufs=2) as sp, \
             tc.tile_pool(name="setup_ps", bufs=2, space="PSUM") as spp:
            sq_scr = None
            for b in range(B):
                for t in range(CT):
                    col = b * CT + t
                    xs = x_all[:, col * s_loc:(col + 1) * s_loc]
                    nc.vector.tensor_reduce(
                        stats_s[:, col:col + 1], xs, mybir.AxisListType.X,
                        mybir.AluOpType.add)
                    sq_scr = sp.tile([128, s_loc], BF16, tag="sq_scr")
                    nc.scalar.activation(
                        sq_scr[:], xs, mybir.ActivationFunctionType.Square,
                        accum_out=stats_s[:, 8 + col:9 + col])

            mask2 = sp.tile([128, 2], F32, tag="mask2")
            nc.vector.memset(mask2[:, :], 0.0)
            nc.vector.memset(mask2[0:64, 0:1], 1.0)
            nc.vector.memset(mask2[64:128, 1:2], 1.0)
            st_p = spp.tile([8, 4], F32, tag="st_p")
            nc.tensor.matmul(st_p[:, 0:2], stats_s[:, 0:8], mask2[:],
                             start=True, stop=True)
            nc.tensor.matmul(st_p[:, 2:4], stats_s[:, 8:16], mask2[:],
                             start=True, stop=True)
            red_s = sp.tile([8, 4], F32, tag="red_s")
            nc.vector.tensor_copy(red_s[:], st_p[:])

            cc_in = dram.tile([8, 4], F32, tag="cc_in")
            cc_out = dram.tile([8, 4], F32, tag="cc_out")
            nc.sync.dma_start(cc_in[:], red_s[:])
            nc.gpsimd.collective_compute(
                "AllReduce", mybir.AluOpType.add,
                replica_groups=[list(range(NCORES))],
                ins=[cc_in.opt()], outs=[cc_out.opt()])
            allst = sp.tile([8, 4], F32, tag="allst")
            nc.sync.dma_start(allst[:], cc_out[:])

            # per-(b,g) mean / rstd  (g = t*2 + half)
            mu8 = sp.tile([8, 2], F32, tag="mu8")
            rstd8 = sp.tile([8, 2], F32, tag="rstd8")
            ex28 = sp.tile([8, 2], F32, tag="ex28")
            var8 = sp.tile([8, 2], F32, tag="var8")
            sd8 = sp.tile([8, 2], F32, tag="sd8")
            eps8 = sp.tile([8, 1], F32, tag="eps8")
            nc.vector.memset(eps8[:], EPS)
            nc.vector.tensor_scalar_mul(mu8[:], allst[:, 0:2], 1.0 / n_gn)
            nc.vector.tensor_scalar_mul(ex28[:], allst[:, 2:4], 1.0 / n_gn)
            nc.vector.scalar_tensor_tensor(
                var8[:], mu8[:], -1.0, mu8[:],
                mybir.AluOpType.mult, mybir.AluOpType.mult)
            nc.vector.tensor_add(var8[:], var8[:], ex28[:])
            nc.scalar.activation(sd8[:], var8[:],
                                 mybir.ActivationFunctionType.Sqrt, bias=eps8[:])
            nc.vector.reciprocal(rstd8[:], sd8[:])

            # broadcast [8,2] -> [128, 8] via DRAM bounce + stride-0 DMA
            mu_d = dram.tile([8, 2], F32, tag="mu_d")
            rstd_d = dram.tile([8, 2], F32, tag="rstd_d")
            nc.sync.dma_start(mu_d[:], mu8[:])
            nc.sync.dma_start(rstd_d[:], rstd8[:])
            mu_bc = sp.tile([128, 8], F32, tag="mu_bc")
            rstd_bc = sp.tile([128, 8], F32, tag="rstd_bc")
            for half in range(2):
                nc.sync.dma_start(
                    mu_bc[half * 64:(half + 1) * 64, :],
                    mu_d[:, half:half + 1].transpose([1, 0]).broadcast_to((64, 8)))
                nc.sync.dma_start(
                    rstd_bc[half * 64:(half + 1) * 64, :],
                    rstd_d[:, half:half + 1].transpose([1, 0]).broadcast_to((64, 8)))

            # per-channel affine: h = a*x + bias
            nc.vector.tensor_mul(a_pc[:], rstd_bc[:], gnw_t[:])
            tmp_bc = sp.tile([128, 8], F32, tag="tmp_bc")
            nc.vector.tensor_mul(tmp_bc[:], mu_bc[:], a_pc[:])
            nc.vector.tensor_sub(bias_pc[:], gnb_t[:], tmp_bc[:])

            # ---- context path: LN + transpose + K/V ----
            ctxf = sp.tile([128, B * LT * CTX], F32, tag="ctxf")
            for b in range(B):
                for lt in range(LT):
                    nc.sync.dma_start(
                        ctxf[:, (b * LT + lt) * CTX:(b * LT + lt + 1) * CTX],
                        ctx_in[b, lt])
            ctxn = sp.tile([128, B * LT * CTX], BF16, tag="ctxn")
            eps128 = sp.tile([128, 1], F32, tag="eps128")
            nc.vector.memset(eps128[:], EPS)
            for b in range(B):
                for lt in range(LT):
                    cs = ctxf[:, (b * LT + lt) * CTX:(b * LT + lt + 1) * CTX]
                    cs1 = sp.tile([128, 1], F32, tag="cs1")
                    cs2 = sp.tile([128, 1], F32, tag="cs2")
                    csq = sp.tile([128, CTX], F32, tag="csq")
                    nc.vector.tensor_reduce(cs1[:], cs, mybir.AxisListType.X,
                                            mybir.AluOpType.add)
                    nc.scalar.activation(csq[:], cs,
                                         mybir.ActivationFunctionType.Square,
                                         accum_out=cs2[:])
                    cmu = sp.tile([128, 1], F32, tag="cmu")
                    cex2 = sp.tile([128, 1], F32, tag="cex2")
                    cvar = sp.tile([128, 1], F32, tag="cvar")
                    csd = sp.tile([128, 1], F32, tag="csd")
                    crstd = sp.tile([128, 1], F32, tag="crstd")
                    cnm = sp.tile([128, 1], F32, tag="cnm")
                    nc.vector.tensor_scalar_mul(cmu[:], cs1[:], 1.0 / CTX)
                    nc.vector.tensor_scalar_mul(cex2[:], cs2[:], 1.0 / CTX)
                    nc.vector.scalar_tensor_tensor(
                        cvar[:], cmu[:], -1.0, cmu[:],
                        mybir.AluOpType.mult, mybir.AluOpType.mult)
                    nc.vector.tensor_add(cvar[:], cvar[:], cex2[:])
                    nc.scalar.activation(csd[:], cvar[:],
                                         mybir.ActivationFunctionType.Sqrt,
                                         bias=eps128[:])
                    nc.vector.reciprocal(crstd[:], csd[:])
                    nc.vector.scalar_tensor_tensor(
                        cnm[:], cmu[:], -1.0, crstd[:],
                        mybir.AluOpType.mult, mybir.AluOpType.mult)
                    nc.vector.tensor_scalar(
                        ctxn[:, (b * LT + lt) * CTX:(b * LT + lt + 1) * CTX],
                        cs, crstd[:], cnm[:],
                        mybir.AluOpType.mult, mybir.AluOpType.add)

            # transpose ctxn -> ctxT_all  [128ctx, L] per (b, kct)
            for b in range(B):
                for lt in range(LT):
                    for ct in range(KCT):
                        tp_p = spp.tile([128, 128], BF16, tag="tp_p")
                        nc.tensor.transpose(
                            tp_p[:],
                            ctxn[:, (b * LT + lt) * CTX + ct * 128:
                                 (b * LT + lt) * CTX + (ct + 1) * 128],
                            ident[:])
                        nc.scalar.copy(
                            ctxT_all[:, (b * KCT + ct) * L + lt * 128:
                                     (b * KCT + ct) * L + (lt + 1) * 128],
                            tp_p[:])

            # kT[b, ot] [128, L]
            for b in range(B):
                for ot in range(OT):
                    k_p = spp.tile([128, L], F32, tag="k_p")
                    for ct in range(KCT):
                        nc.tensor.matmul(
                            k_p[:],
                            kw_t[:, ct * C + ot * 128:ct * C + (ot + 1) * 128],
                            ctxT_all[:, (b * KCT + ct) * L:(b * KCT + ct + 1) * L],
                            start=(ct == 0), stop=(ct == KCT - 1))
                    nc.scalar.activation(
                        kT_all[:, (b * CT + ot) * L:(b * CT + ot + 1) * L],
                        k_p[:], mybir.ActivationFunctionType.Identity,
                        bias=kb_t[:, ot:ot + 1])

            # v'[b, lt] [128, NH*(HD+1)]  (per-head ones column appended)
            VW = NH * (HD + 1)
            for b in range(B):
                for lt in range(LT):
                    v_p = spp.tile([128, C], F32, tag="v_p")
                    for ct in range(KCT):
                        nc.tensor.matmul(
                            v_p[:],
                            ctxT_all[:, (b * KCT + ct) * L + lt * 128:
                                     (b * KCT + ct) * L + (lt + 1) * 128],
                            vw_t[:, ct * C:(ct + 1) * C],
                            start=(ct == 0), stop=(ct == KCT - 1))
                    vs = v_all[:, (b * LT + lt) * VW:(b * LT + lt + 1) * VW]
                    nc.scalar.copy(
                        vs.rearrange("p (h e) -> p h e", e=HD + 1)[:, :, 0:HD],
                        v_p[:])
                    nc.vector.memset(
                        vs.rearrange("p (h e) -> p h e", e=HD + 1)[:, :, HD:HD + 1],
                        1.0)

        # ---- main attention loop ----
        with tc.tile_pool(name="mp", bufs=2) as mp, \
             tc.tile_pool(name="op", bufs=3) as op, \
             tc.tile_pool(name="mm_ps", bufs=2, space="PSUM") as mmp, \
             tc.tile_pool(name="z_ps", bufs=2, space="PSUM") as zp, \
             tc.tile_pool(name="o_ps", bufs=2, space="PSUM") as opp, \
             tc.tile_pool(name="rb_ps", bufs=2, space="PSUM") as rbp:
            VW = NH * (HD + 1)
            for b in range(B):
                for st in range(nst):
                    lo = st * st_size
                    # h = a*x + bias  (bf16), per channel-tile
                    h_ts = []
                    for ct in range(CT):
                        col = b * CT + ct
                        h_t = mp.tile([128, st_size], BF16, tag=f"h{ct}")
                        nc.vector.tensor_scalar(
                            h_t[:],
                            x_all[:, col * s_loc + lo:col * s_loc + lo + st_size],
                            a_pc[:, col:col + 1], bias_pc[:, col:col + 1],
                            mybir.AluOpType.mult, mybir.AluOpType.add)
                        h_ts.append(h_t)
                    # q
                    q_s = mp.tile([128, CT * st_size], BF16, tag="q_s")
                    for ot in range(OT):
                        q_p = mmp.tile([128, st_size], F32, tag="mm_p")
                        for ct in range(CT):
                            nc.tensor.matmul(
                                q_p[:],
                                qw_t[:, ct * C + ot * 128:ct * C + (ot + 1) * 128],
                                h_ts[ct][:],
                                start=(ct == 0), stop=(ct == CT - 1))
                        nc.scalar.activation(
                            q_s[:, ot * st_size:(ot + 1) * st_size], q_p[:],
                            mybir.ActivationFunctionType.Identity,
                            bias=qb_t[:, ot:ot + 1])
                    # heads
                    proj_rhs = mp.tile([128, CT * st_size], BF16, tag="proj_rhs")
                    for hh in range(NH):
                        ko, po = hh // 2, (hh % 2) * 64
                        p_t = mp.tile([128, 2 * st_size], BF16, tag="p_t")
                        for lh in range(LT):
                            z_p = zp.tile([128, st_size], F32, tag="z_p")
                            nc.tensor.matmul(
                                z_p[:],
                                kT_all[po:po + 64,
                                       (b * CT + ko) * L + lh * 128:
                                       (b * CT + ko) * L + (lh + 1) * 128],
                                q_s[po:po + 64, ko * st_size:(ko + 1) * st_size],
                                start=True, stop=True)
                            nc.scalar.activation(
                                p_t[:, lh * st_size:(lh + 1) * st_size], z_p[:],
                                mybir.ActivationFunctionType.Exp)
                        o_p = opp.tile([HD + 1, st_size], F32, tag="o_p")
                        for lh in range(LT):
                            nc.tensor.matmul(
                                o_p[:],
                                v_all[:, (b * LT + lh) * VW + hh * (HD + 1):
                                      (b * LT + lh) * VW + (hh + 1) * (HD + 1)],
                                p_t[:, lh * st_size:(lh + 1) * st_size],
                                start=(lh == 0), stop=(lh == LT - 1))
                        rs_s = mp.tile([1, st_size], F32, tag="rs_s")
                        nc.vector.reciprocal(rs_s[:], o_p[HD:HD + 1, :])
                        rb_p = rbp.tile([64, st_size], F32, tag="rb_p")
                        nc.tensor.matmul(rb_p[:], ones_t[:], rs_s[:],
                                         start=True, stop=True)
                        rb_s = mp.tile([64, st_size], F32, tag="rb_s")
                        nc.scalar.copy(rb_s[:], rb_p[:])
                        nc.vector.tensor_tensor(
                            proj_rhs[po:po + 64, ko * st_size:(ko + 1) * st_size],
                            o_p[0:HD, :], rb_s[:], mybir.AluOpType.mult)
                    # proj + residual
                    for ot in range(OT):
                        y_p = mmp.tile([128, st_size], F32, tag="mm_p")
                        for ct in range(CT):
                            nc.tensor.matmul(
                                y_p[:],
                                pw_t[:, ct * C + ot * 128:ct * C + (ot + 1) * 128],
                                proj_rhs[:, ct * st_size:(ct + 1) * st_size],
                                start=(ct == 0), stop=(ct == CT - 1))
                        col = b * CT + ot
                        out_s = op.tile([128, st_size], F32, tag="out_s")
                        nc.vector.scalar_tensor_tensor(
                            out_s[:], y_p[:], pb_t[:, ot:ot + 1],
                            x_all[:, col * s_loc + lo:col * s_loc + lo + st_size],
                            mybir.AluOpType.add, mybir.AluOpType.add)
                        nc.sync.dma_start(out_ext[b, ot, :, lo:lo + st_size],
                                          out_s[:])
    return nc


def prep_inputs(x, context, gn_w, gn_b, ln_w, ln_b, q_w, q_b, k_w, k_b,
                v_w, v_b, proj_w, proj_b, s_loc):
    """Host-side shard + layout prep. Returns in_maps for the 8 cores."""
    scale = HD ** -0.5
    qwT = (q_w.astype(np.float64) * scale).T.astype(np.float32)
    kwT = (k_w.astype(np.float64) * ln_w.astype(np.float64)[None, :]).T.astype(np.float32)
    vwT = (v_w.astype(np.float64) * ln_w.astype(np.float64)[None, :]).T.astype(np.float32)
    pwT = proj_w.T.astype(np.float32)
    kb_eff = (k_b + ln_b @ k_w.T).astype(np.float32)
    vb_eff = (v_b + ln_b @ v_w.T).astype(np.float32)
    pb_eff = (proj_b + vb_eff @ proj_w.T).astype(np.float32)
    qb_eff = (q_b * scale).astype(np.float32)

    gnw8 = np.empty((128, B * CT), np.float32)
    gnb8 = np.empty((128, B * CT), np.float32)
    for b in range(B):
        for t in range(CT):
            gnw8[:, b * CT + t] = gn_w[t * 128:(t + 1) * 128]
            gnb8[:, b * CT + t] = gn_b[t * 128:(t + 1) * 128]

    shared = {
        "ctx": np.ascontiguousarray(context.reshape(B, LT, 128, CTX)).astype(np.float32),
        "qwT": np.ascontiguousarray(qwT.reshape(CT, 128, C)).astype(BF),
        "kwT": np.ascontiguousarray(kwT.reshape(KCT, 128, C)).astype(BF),
        "vwT": np.ascontiguousarray(vwT.reshape(KCT, 128, C)).astype(BF),
        "pwT": np.ascontiguousarray(pwT.reshape(CT, 128, C)).astype(BF),
        "qb": np.ascontiguousarray(qb_eff.reshape(CT, 128, 1)),
        "kb": np.ascontiguousarray(kb_eff.reshape(CT, 128, 1)),
        "pb": np.ascontiguousarray(pb_eff.reshape(CT, 128, 1)),
        "gnw8": gnw8, "gnb8": gnb8,
    }
    xr = x.reshape(B, C, S)
    in_maps = []
    for i in range(NCORES):
        xs = np.ascontiguousarray(xr[:, :, i * s_loc:(i + 1) * s_loc])
        m = dict(shared)
        m["x"] = xs.reshape(B, CT, 128, s_loc).astype(BF)
        in_maps.append(m)
    return in_maps


def _install_prof_shim():
    """Register the NTFF profile hook that this container's antenv lacks."""
    import types
    import antenv

    if "antenv.axon_hooks" not in sys.modules:
        mod = types.ModuleType("antenv.axon_hooks")
        mod._hook = None
        mod.set_axon_ntff_profile_hook = lambda h: setattr(mod, "_hook", h)
        mod.get_axon_ntff_profile_hook = lambda: mod._hook
        sys.modules["antenv.axon_hooks"] = mod
        antenv.axon_hooks = mod
    sys.path.insert(0, "/root/.axon_site")
    from trn_agent_boot.trn_boot import _ntff_profile_via_ctypes
    from antenv.axon_hooks import set_axon_ntff_profile_hook

    hook = _ntff_profile_via_ctypes("/opt/axon/libaxon_pjrt.so")
    assert hook is not None
    set_axon_ntff_profile_hook(hook)
    from concourse import bass_utils as bu
    bu.upload_artifacts = lambda tmpdir: tmpdir


def kernel(x, context, gn_w, gn_b, ln_w, ln_b, q_w, q_b, k_w, k_b,
           v_w, v_b, proj_w, proj_b):
    import os
    s_loc = S // NCORES
    st_size = 512
    in_maps = prep_inputs(x, context, gn_w, gn_b, ln_w, ln_b, q_w, q_b,
                          k_w, k_b, v_w, v_b, proj_w, proj_b, s_loc)
    nc = bacc.Bacc("TRN2", target_bir_lowering=False, debug=False,
                   num_devices=NCORES)
    build(nc, s_loc, st_size)
    nc.compile()
    trace = bool(os.environ.get("KPROF"))
    if trace:
        try:
            _install_prof_shim()
        except Exception as e:
            print(f"profiling shim unavailable ({e}); running untraced")
            trace = False
    try:
        res = run_bass_kernel_spmd(nc, in_maps, list(range(NCORES)),
                                   trace=trace,
                                   tmpdir=os.environ.get("KPROF_DIR"))
    except Exception:
        if not trace:
            raise
        print("traced run failed; retrying untraced")
        res = run_bass_kernel_spmd(nc, in_maps, list(range(NCORES)))
    if trace and res.exec_time_ns is not None:
        print(f"HW exec time: {res.exec_time_ns} ns")
    out = np.empty((B, C, S), np.float32)
    for i in range(NCORES):
        out[:, :, i * (S // NCORES):(i + 1) * (S // NCORES)] = \
            res.results[i]["out"].reshape(B, C, S // NCORES)
    return out.reshape(B, C, D, H, W)


# revision 17
# speedup vs baseline: 1.3788x; 1.3788x over previous
"""CrossAttentionBlock3D on 8 TRN2 NeuronCores — sequence-parallel Bass kernel.

Sharding: the 32768 spatial tokens are split 8x4096 across cores. GroupNorm
statistics are the only cross-core dependency (one 64-float AllReduce).
Everything else (LN, K/V projections over the tiny context, Q/attention/proj
for the local tokens) is computed locally; context-side work is replicated.

Compute dtype: bf16 matmuls with f32 PSUM accumulation (validated end-to-end
rel err ~3e-3 vs the f32 reference).
"""
import sys

sys.path.insert(0, "/opt/trn_rl_repo")

import numpy as np
import ml_dtypes

from concourse import bass, bacc, tile, mybir, masks
from concourse.bass_utils import run_bass_kernel_spmd

F32 = mybir.dt.float32
BF16 = mybir.dt.bfloat16
BF = ml_dtypes.bfloat16

B, C, D, H, W = 2, 512, 32, 32, 32
S = D * H * W              # 32768
L, CTX = 256, 768
NH, HD, G = 8, 64, 8
EPS = 1e-5
NCORES = 8
CT, OT = 4, 4              # channel tiles (C = 4*128)
KCT = 6                    # ctx channel tiles (CTX = 6*128)
LT = 2                     # L = 2*128


def build(nc, s_loc, st_size):
    """Emit the per-core Tile program. s_loc: local tokens; st_size: S-tile."""
    nst = s_loc // st_size
    n_gn = (C // G) * S    # global elems per (b, g) group

    x_in = nc.dram_tensor("x", [B, CT, 128, s_loc], BF16, kind="ExternalInput")
    ctx_in = nc.dram_tensor("ctx", [B, LT, 128, CTX], F32, kind="ExternalInput")
    qw_in = nc.dram_tensor("qwT", [CT, 128, C], BF16, kind="ExternalInput")
    kw_in = nc.dram_tensor("kwT", [KCT, 128, C], BF16, kind="ExternalInput")
    vw_in = nc.dram_tensor("vwT", [KCT, 128, C], BF16, kind="ExternalInput")
    pw_in = nc.dram_tensor("pwT", [CT, 128, C], BF16, kind="ExternalInput")
    qb_in = nc.dram_tensor("qb", [CT, 128, 1], F32, kind="ExternalInput")
    kb_in = nc.dram_tensor("kb", [CT, 128, 1], F32, kind="ExternalInput")
    pb_in = nc.dram_tensor("pb", [CT, 128, 1], F32, kind="ExternalInput")
    gnw_in = nc.dram_tensor("gnw8", [128, B * CT], F32, kind="ExternalInput")
    gnb_in = nc.dram_tensor("gnb8", [128, B * CT], F32, kind="ExternalInput")
    out_ext = nc.dram_tensor("out", [B, CT, 128, s_loc], F32, kind="ExternalOutput")

    from contextlib import ExitStack
    with tile.TileContext(nc) as tc, ExitStack() as es:
        wp = es.enter_context(tc.tile_pool(name="wp", bufs=1))
        dram = es.enter_context(tc.tile_pool(name="dram", bufs=1, space="DRAM"))

        # ---- persistent SBUF tensors ----
        x_all = wp.tile([128, B * CT * s_loc], BF16, tag="x_all")
        qw_t = wp.tile([128, CT * C], BF16, tag="qw")
        kw_t = wp.tile([128, KCT * C], BF16, tag="kw")
        vw_t = wp.tile([128, KCT * C], BF16, tag="vw")
        pw_t = wp.tile([128, CT * C], BF16, tag="pw")
        qb_t = wp.tile([128, CT], F32, tag="qb")
        kb_t = wp.tile([128, CT], F32, tag="kb")
        pb_t = wp.tile([128, CT], F32, tag="pb")
        gnw_t = wp.tile([128, B * CT], F32, tag="gnw")
        gnb_t = wp.tile([128, B * CT], F32, tag="gnb")
        ctxT_all = wp.tile([128, B * KCT * L], BF16, tag="ctxT")
        kT_all = wp.tile([128, B * CT * L], BF16, tag="kT")
        v_all = wp.tile([128, B * LT * (NH * (HD + 1))], BF16, tag="v")
        ones_t = wp.tile([1, 64], F32, tag="ones")
        ident = wp.tile([128, 128], BF16, tag="ident")
        stats_s = wp.tile([128, 16], F32, tag="stats")
        a_pc = wp.tile([128, B * CT], F32, tag="a_pc")
        bias_pc = wp.tile([128, B * CT], F32, tag="bias_pc")

        for b in range(B):
            for t in range(CT):
                nc.sync.dma_start(
                    x_all[:, (b * CT + t) * s_loc:(b * CT + t + 1) * s_loc],
                    x_in[b, t],
                )
        for t in range(CT):
            nc.sync.dma_start(qw_t[:, t * C:(t + 1) * C], qw_in[t])
            nc.sync.dma_start(pw_t[:, t * C:(t + 1) * C], pw_in[t])
            nc.sync.dma_start(qb_t[:, t:t + 1], qb_in[t])
            nc.sync.dma_start(kb_t[:, t:t + 1], kb_in[t])
            nc.sync.dma_start(pb_t[:, t:t + 1], pb_in[t])
        for t in range(KCT):
            nc.sync.dma_start(kw_t[:, t * C:(t + 1) * C], kw_in[t])
            nc.sync.dma_start(vw_t[:, t * C:(t + 1) * C], vw_in[t])
        nc.sync.dma_start(gnw_t[:], gnw_in[:])
        nc.sync.dma_start(gnb_t[:], gnb_in[:])
        nc.vector.memset(ones_t[:], 1.0)
        masks.make_identity(nc, ident[:])

        # ---- GroupNorm partial stats (local) ----
        with tc.tile_pool(name="setup", bufs=2) as sp, \
             tc.tile_pool(name="setup_ps", bufs=2, space="PSUM") as spp:
            sq_scr = None
            for b in range(B):
                for t in range(CT):
                    col = b * CT + t
                    xs = x_all[:, col * s_loc:(col + 1) * s_loc]
                    nc.vector.tensor_reduce(
                        stats_s[:, col:col + 1], xs, mybir.AxisListType.X,
                        mybir.AluOpType.add)
                    sq_scr = sp.tile([128, s_loc], BF16, tag="sq_scr")
                    nc.scalar.activation(
                        sq_scr[:], xs, mybir.ActivationFunctionType.Square,
                        accum_out=stats_s[:, 8 + col:9 + col])

            mask2 = sp.tile([128, 2], F32, tag="mask2")
            nc.vector.memset(mask2[:, :], 0.0)
            nc.vector.memset(mask2[0:64, 0:1], 1.0)
            nc.vector.memset(mask2[64:128, 1:2], 1.0)
            st_p = spp.tile([8, 4], F32, tag="st_p")
            nc.tensor.matmul(st_p[:, 0:2], stats_s[:, 0:8], mask2[:],
                             start=True, stop=True)
            nc.tensor.matmul(st_p[:, 2:4], stats_s[:, 8:16], mask2[:],
                             start=True, stop=True)
            red_s = sp.tile([8, 4], F32, tag="red_s")
            nc.vector.tensor_copy(red_s[:], st_p[:])

            cc_in = dram.tile([8, 4], F32, tag="cc_in")
            cc_out = dram.tile([8, 4], F32, tag="cc_out")
            nc.sync.dma_start(cc_in[:], red_s[:])
            nc.gpsimd.collective_compute(
                "AllReduce", mybir.AluOpType.add,
                replica_groups=[list(range(NCORES))],
                ins=[cc_in.opt()], outs=[cc_out.opt()])
            allst = sp.tile([8, 4], F32, tag="allst")
            nc.sync.dma_start(allst[:], cc_out[:])

            # per-(b,g) mean / rstd  (g = t*2 + half)
            mu8 = sp.tile([8, 2], F32, tag="mu8")
            rstd8 = sp.tile([8, 2], F32, tag="rstd8")
            ex28 = sp.tile([8, 2], F32, tag="ex28")
            var8 = sp.tile([8, 2], F32, tag="var8")
            sd8 = sp.tile([8, 2], F32, tag="sd8")
            eps8 = sp.tile([8, 1], F32, tag="eps8")
            nc.vector.memset(eps8[:], EPS)
            nc.vector.tensor_scalar_mul(mu8[:], allst[:, 0:2], 1.0 / n_gn)
            nc.vector.tensor_scalar_mul(ex28[:], allst[:, 2:4], 1.0 / n_gn)
            nc.vector.scalar_tensor_tensor(
                var8[:], mu8[:], -1.0, mu8[:],
                mybir.AluOpType.mult, mybir.AluOpType.mult)
            nc.vector.tensor_add(var8[:], var8[:], ex28[:])
            nc.scalar.activation(sd8[:], var8[:],
                                 mybir.ActivationFunctionType.Sqrt, bias=eps8[:])
            nc.vector.reciprocal(rstd8[:], sd8[:])

            # broadcast [8,2] -> [128, 8] via DRAM bounce + stride-0 DMA
            mu_d = dram.tile([8, 2], F32, tag="mu_d")
            rstd_d = dram.tile([8, 2], F32, tag="rstd_d")
            nc.sync.dma_start(mu_d[:], mu8[:])
            nc.sync.dma_start(rstd_d[:], rstd8[:])
            mu_bc = sp.tile([128, 8], F32, tag="mu_bc")
            rstd_bc = sp.tile([128, 8], F32, tag="rstd_bc")
            for half in range(2):
                nc.sync.dma_start(
                    mu_bc[half * 64:(half + 1) * 64, :],
                    mu_d[:, half:half + 1].transpose([1, 0]).broadcast_to((64, 8)))
                nc.sync.dma_start(
                    rstd_bc[half * 64:(half + 1) * 64, :],
                    rstd_d[:, half:half + 1].transpose([1, 0]).broadcast_to((64, 8)))

            # per-channel affine: h = a*x + bias
            nc.vector.tensor_mul(a_pc[:], rstd_bc[:], gnw_t[:])
            tmp_bc = sp.tile([128, 8], F32, tag="tmp_bc")
            nc.vector.tensor_mul(tmp_bc[:], mu_bc[:], a_pc[:])
            nc.vector.tensor_sub(bias_pc[:], gnb_t[:], tmp_bc[:])

            # ---- context path: LN + transpose + K/V ----
            ctxf = sp.tile([128, B * LT * CTX], F32, tag="ctxf")
            for b in range(B):
                for lt in range(LT):
                    nc.sync.dma_start(
                        ctxf[:, (b * LT + lt) * CTX:(b * LT + lt + 1) * CTX],
                        ctx_in[b, lt])
            ctxn = sp.tile([128, B * LT * CTX], BF16, tag="ctxn")
            eps128 = sp.tile([128, 1], F32, tag="eps128")
            nc.vector.memset(eps128[:], EPS)
            for b in range(B):
                for lt in range(LT):
                    cs = ctxf[:, (b * LT + lt) * CTX:(b * LT + lt + 1) * CTX]
                    cs1 = sp.tile([128, 1], F32, tag="cs1")
                    cs2 = sp.tile([128, 1], F32, tag="cs2")
                    csq = sp.tile([128, CTX], F32, tag="csq")
                    nc.vector.tensor_reduce(cs1[:], cs, mybir.AxisListType.X,
                                            mybir.AluOpType.add)
                    nc.scalar.activation(csq[:], cs,
                                         mybir.ActivationFunctionType.Square,
                                         accum_out=cs2[:])
                    cmu = sp.tile([128, 1], F32, tag="cmu")
                    cex2 = sp.tile([128, 1], F32, tag="cex2")
                    cvar = sp.tile([128, 1], F32, tag="cvar")
                    csd = sp.tile([128, 1], F32, tag="csd")
                    crstd = sp.tile([128, 1], F32, tag="crstd")
                    cnm = sp.tile([128, 1], F32, tag="cnm")
                    nc.vector.tensor_scalar_mul(cmu[:], cs1[:], 1.0 / CTX)
                    nc.vector.tensor_scalar_mul(cex2[:], cs2[:], 1.0 / CTX)
                    nc.vector.scalar_tensor_tensor(
                        cvar[:], cmu[:], -1.0, cmu[:],
                        mybir.AluOpType.mult, mybir.AluOpType.mult)
                    nc.vector.tensor_add(cvar[:], cvar[:], cex2[:])
                    nc.scalar.activation(csd[:], cvar[:],
                                         mybir.ActivationFunctionType.Sqrt,
                                         bias=eps128[:])
                    nc.vector.reciprocal(crstd[:], csd[:])
                    nc.vector.scalar_tensor_tensor(
                        cnm[:], cmu[:], -1.0, crstd[:],
                        mybir.AluOpType.mult, mybir.AluOpType.mult)
                    nc.vector.tensor_scalar(
                        ctxn[:, (b * LT + lt) * CTX:(b * LT + lt + 1) * CTX],
                        cs, crstd[:], cnm[:],
                        mybir.AluOpType.mult, mybir.AluOpType.add)

            # transpose ctxn -> ctxT_all  [128ctx, L] per (b, kct)
            for b in range(B):
                for lt in range(LT):
                    for ct in range(KCT):
                        tp_p = spp.tile([128, 128], BF16, tag="tp_p")
                        nc.tensor.transpose(
                            tp_p[:],
                            ctxn[:, (b * LT + lt) * CTX + ct * 128:
                                 (b * LT + lt) * CTX + (ct + 1) * 128],
                            ident[:])
                        nc.scalar.copy(
                            ctxT_all[:, (b * KCT + ct) * L + lt * 128:
                                     (b * KCT + ct) * L + (lt + 1) * 128],
                            tp_p[:])

            # kT[b, ot] [128, L]
            for b in range(B):
                for ot in range(OT):
                    k_p = spp.tile([128, L], F32, tag="k_p")
                    for ct in range(KCT):
                        nc.tensor.matmul(
                            k_p[:],
                            kw_t[:, ct * C + ot * 128:ct * C + (ot + 1) * 128],
                            ctxT_all[:, (b * KCT + ct) * L:(b * KCT + ct + 1) * L],
                            start=(ct == 0), stop=(ct == KCT - 1))
                    nc.scalar.activation(
                        kT_all[:, (b * CT + ot) * L:(b * CT + ot + 1) * L],
                        k_p[:], mybir.ActivationFunctionType.Identity,
                        bias=kb_t[:, ot:ot + 1])

            # v'[b, lt] [128, NH*(HD+1)]  (per-head ones column appended)
            VW = NH * (HD + 1)
            for b in range(B):
                for lt in range(LT):
                    v_p = spp.tile([128, C], F32, tag="v_p")
                    for ct in range(KCT):
                        nc.tensor.matmul(
                            v_p[:],
                            ctxT_all[:, (b * KCT + ct) * L + lt * 128:
                                     (b * KCT + ct) * L + (lt + 1) * 128],
                            vw_t[:, ct * C:(ct + 1) * C],
                            start=(ct == 0), stop=(ct == KCT - 1))
                    vs = v_all[:, (b * LT + lt) * VW:(b * LT + lt + 1) * VW]
                    nc.scalar.copy(
                        vs.rearrange("p (h e) -> p h e", e=HD + 1)[:, :, 0:HD],
                        v_p[:])
                    nc.vector.memset(
                        vs.rearrange("p (h e) -> p h e", e=HD + 1)[:, :, HD:HD + 1],
                        1.0)

        # ---- main attention loop ----
        with tc.tile_pool(name="mp", bufs=2) as mp, \
             tc.tile_pool(name="op", bufs=3) as op, \
             tc.tile_pool(name="mm_ps", bufs=2, space="PSUM") as mmp, \
             tc.tile_pool(name="z_ps", bufs=2, space="PSUM") as zp, \
             tc.tile_pool(name="o_ps", bufs=3, space="PSUM") as opp, \
             tc.tile_pool(name="rs_dram", bufs=4, space="DRAM") as rsd:
            VW = NH * (HD + 1)
            for b in range(B):
                for st in range(nst):
                    lo = st * st_size
                    # h = a*x + bias  (bf16), per channel-tile
                    h_ts = []
                    for ct in range(CT):
                        col = b * CT + ct
                        h_t = mp.tile([128, st_size], BF16, tag=f"h{ct}")
                        nc.vector.tensor_scalar(
                            h_t[:],
                            x_all[:, col * s_loc + lo:col * s_loc + lo + st_size],
                            a_pc[:, col:col + 1], bias_pc[:, col:col + 1],
                            mybir.AluOpType.mult, mybir.AluOpType.add)
                        h_ts.append(h_t)
                    # q
                    q_s = mp.tile([128, CT * st_size], BF16, tag="q_s")
                    for ot in range(OT):
                        q_p = mmp.tile([128, st_size], F32, tag="mm_p")
                        for ct in range(CT):
                            nc.tensor.matmul(
                                q_p[:],
                                qw_t[:, ct * C + ot * 128:ct * C + (ot + 1) * 128],
                                h_ts[ct][:],
                                start=(ct == 0), stop=(ct == CT - 1))
                        nc.scalar.activation(
                            q_s[:, ot * st_size:(ot + 1) * st_size], q_p[:],
                            mybir.ActivationFunctionType.Identity,
                            bias=qb_t[:, ot:ot + 1])
                    # heads
                    proj_rhs = mp.tile([128, CT * st_size], BF16, tag="proj_rhs")
                    for hh in range(NH):
                        ko, po = hh // 2, (hh % 2) * 64
                        p_t = mp.tile([128, 2 * st_size], BF16, tag="p_t")
                        for lh in range(LT):
                            z_p = zp.tile([128, st_size], F32, tag="z_p")
                            nc.tensor.matmul(
                                z_p[:],
                                kT_all[po:po + 64,
                                       (b * CT + ko) * L + lh * 128:
                                       (b * CT + ko) * L + (lh + 1) * 128],
                                q_s[po:po + 64, ko * st_size:(ko + 1) * st_size],
                                start=True, stop=True)
                            nc.scalar.activation(
                                p_t[:, lh * st_size:(lh + 1) * st_size], z_p[:],
                                mybir.ActivationFunctionType.Exp)
                        o_p = opp.tile([HD + 1, st_size], F32, tag="o_p")
                        for lh in range(LT):
                            nc.tensor.matmul(
                                o_p[:],
                                v_all[:, (b * LT + lh) * VW + hh * (HD + 1):
                                      (b * LT + lh) * VW + (hh + 1) * (HD + 1)],
                                p_t[:, lh * st_size:(lh + 1) * st_size],
                                start=(lh == 0), stop=(lh == LT - 1))
                        rs_s = mp.tile([1, st_size], F32, tag="rs_s")
                        nc.vector.reciprocal(rs_s[:], o_p[HD:HD + 1, :])
                        rs_d = rsd.tile([1, st_size], F32, tag="rs_d")
                        nc.sync.dma_start(rs_d[:], rs_s[:])
                        rb_s = mp.tile([64, st_size], F32, tag="rb_s")
                        nc.sync.dma_start(
                            rb_s[:], rs_d[0:1, :].broadcast_to((64, st_size)))
                        nc.vector.tensor_tensor(
                            proj_rhs[po:po + 64, ko * st_size:(ko + 1) * st_size],
                            o_p[0:HD, :], rb_s[:], mybir.AluOpType.mult)
                    # proj + residual
                    for ot in range(OT):
                        y_p = mmp.tile([128, st_size], F32, tag="mm_p")
                        for ct in range(CT):
                            nc.tensor.matmul(
                                y_p[:],
                                pw_t[:, ct * C + ot * 128:ct * C + (ot + 1) * 128],
                                proj_rhs[:, ct * st_size:(ct + 1) * st_size],
                                start=(ct == 0), stop=(ct == CT - 1))
                        col = b * CT + ot
                        out_s = op.tile([128, st_size], F32, tag="out_s")
                        nc.vector.scalar_tensor_tensor(
                            out_s[:], y_p[:], pb_t[:, ot:ot + 1],
                            x_all[:, col * s_loc + lo:col * s_loc + lo + st_size],
                            mybir.AluOpType.add, mybir.AluOpType.add)
                        nc.sync.dma_start(out_ext[b, ot, :, lo:lo + st_size],
                                          out_s[:])
    return nc


def prep_inputs(x, context, gn_w, gn_b, ln_w, ln_b, q_w, q_b, k_w, k_b,
                v_w, v_b, proj_w, proj_b, s_loc):
    """Host-side shard + layout prep. Returns in_maps for the 8 cores."""
    scale = HD ** -0.5
    qwT = (q_w.astype(np.float64) * scale).T.astype(np.float32)
    kwT = (k_w.astype(np.float64) * ln_w.astype(np.float64)[None, :]).T.astype(np.float32)
    vwT = (v_w.astype(np.float64) * ln_w.astype(np.float64)[None, :]).T.astype(np.float32)
    pwT = proj_w.T.astype(np.float32)
    kb_eff = (k_b + ln_b @ k_w.T).astype(np.float32)
    vb_eff = (v_b + ln_b @ v_w.T).astype(np.float32)
    pb_eff = (proj_b + vb_eff @ proj_w.T).astype(np.float32)
    qb_eff = (q_b * scale).astype(np.float32)

    gnw8 = np.empty((128, B * CT), np.float32)
    gnb8 = np.empty((128, B * CT), np.float32)
    for b in range(B):
        for t in range(CT):
            gnw8[:, b * CT + t] = gn_w[t * 128:(t + 1) * 128]
            gnb8[:, b * CT + t] = gn_b[t * 128:(t + 1) * 128]

    shared = {
        "ctx": np.ascontiguousarray(context.reshape(B, LT, 128, CTX)).astype(np.float32),
        "qwT": np.ascontiguousarray(qwT.reshape(CT, 128, C)).astype(BF),
        "kwT": np.ascontiguousarray(kwT.reshape(KCT, 128, C)).astype(BF),
        "vwT": np.ascontiguousarray(vwT.reshape(KCT, 128, C)).astype(BF),
        "pwT": np.ascontiguousarray(pwT.reshape(CT, 128, C)).astype(BF),
        "qb": np.ascontiguousarray(qb_eff.reshape(CT, 128, 1)),
        "kb": np.ascontiguousarray(kb_eff.reshape(CT, 128, 1)),
        "pb": np.ascontiguousarray(pb_eff.reshape(CT, 128, 1)),
        "gnw8": gnw8, "gnb8": gnb8,
    }
    xr = x.reshape(B, C, S)
    in_maps = []
    for i in range(NCORES):
        xs = np.ascontiguousarray(xr[:, :, i * s_loc:(i + 1) * s_loc])
        m = dict(shared)
        m["x"] = xs.reshape(B, CT, 128, s_loc).astype(BF)
        in_maps.append(m)
    return in_maps


def _install_prof_shim():
    """Register the NTFF profile hook that this container's antenv lacks."""
    import types
    import antenv

    if "antenv.axon_hooks" not in sys.modules:
        mod = types.ModuleType("antenv.axon_hooks")
        mod._hook = None
        mod.set_axon_ntff_profile_hook = lambda h: setattr(mod, "_hook", h)
        mod.get_axon_ntff_profile_hook = lambda: mod._hook
        sys.modules["antenv.axon_hooks"] = mod
        antenv.axon_hooks = mod
    sys.path.insert(0, "/root/.axon_site")
    from trn_agent_boot.trn_boot import _ntff_profile_via_ctypes
    from antenv.axon_hooks import set_axon_ntff_profile_hook

    hook = _ntff_profile_via_ctypes("/opt/axon/libaxon_pjrt.so")
    assert hook is not None
    set_axon_ntff_profile_hook(hook)
    from concourse import bass_utils as bu
    bu.upload_artifacts = lambda tmpdir: tmpdir


def kernel(x, context, gn_w, gn_b, ln_w, ln_b, q_w, q_b, k_w, k_b,
           v_w, v_b, proj_w, proj_b):
    import os
    s_loc = S // NCORES
    st_size = 512
    in_maps = prep_inputs(x, context, gn_w, gn_b, ln_w, ln_b, q_w, q_b,
                          k_w, k_b, v_w, v_b, proj_w, proj_b, s_loc)
    nc = bacc.Bacc("TRN2", target_bir_lowering=False, debug=False,
                   num_devices=NCORES)
    build(nc, s_loc, st_size)
    nc.compile()
    trace = bool(os.environ.get("KPROF"))
    if trace:
        try:
            _install_prof_shim()
        except Exception as e:
            print(f"profiling shim unavailable ({e}); running untraced")
            trace = False
    try:
        res = run_bass_kernel_spmd(nc, in_maps, list(range(NCORES)),
                                   trace=trace,
                                   tmpdir=os.environ.get("KPROF_DIR"))
    except Exception:
        if not trace:
            raise
        print("traced run failed; retrying untraced")
        res = run_bass_kernel_spmd(nc, in_maps, list(range(NCORES)))
    if trace and res.exec_time_ns is not None:
        print(f"HW exec time: {res.exec_time_ns} ns")
    out = np.empty((B, C, S), np.float32)
    for i in range(NCORES):
        out[:, :, i * (S // NCORES):(i + 1) * (S // NCORES)] = \
            res.results[i]["out"].reshape(B, C, S // NCORES)
    return out.reshape(B, C, D, H, W)
